# revision 50
# baseline (speedup 1.0000x reference)
"""Trainium2 Bass kernel for nn_Embed_38766374814290 (embedding_lookup).

Math: out[i,j,l,e] = A[m][e] + delta_s[i,j,l] * B[m][e]
  where m = (j < traj_len[i]), delta_s = where(m, mat2[traj_loc-1], 0),
  A[m] = emb_sl_w[m] + emb_tl_w[m],
  B[m] = (emb_su_w[m]-emb_sl_w[m])/SU + (emb_tu_w[m]-emb_tl_w[m])/TU.

Sharding: pure data parallel over batch N = 32 -> 4 rows per core x 8 cores.

Design (bf16 output; the rel-err gate is 2e-2, this kernel lands ~4.4e-3):
  - bf16 output halves the HBM write bytes (16MiB -> 8MiB/core); HBM
    write bandwidth is the binding roofline for this problem.
  - Gather table m2e[4097, 4, 36]: per l-group 32 mat2 bf16 columns plus
    const columns [m, m, 1, 1] baked in (the appended zero row keeps the
    1s), so one contiguous indirect DMA per batch row yields
    transpose-ready tiles with mask handling free. The indirect-DMA out
    AP must be 2-D - a 3-D tile view scatters the gathered rows.
  - The kernel is sized for a COLD (1.2 GHz) PE: HAM never sustains 8/8
    for this burst pattern, so l-groups are processed in PAIRS with
    tile_position row concurrency (even l-group in PE rows 0-63, odd in
    rows 64-127; rhs duplicated at partition 64). The two matmuls of a
    slot write the two halves (= two banks) of ONE [128,1024] PSUM tile
    so the bufs=3 pool never starves a pair.
  - Evictions (f32->bf16, the other throughput wall: PSUM reads are
    1 elem/lane/cycle) go per slot to strided [lg-pair, s] orow
    destinations, alternating DVE 15 / ACT 17 (GPSIMD cannot touch PSUM).
  - Output DMA chunks graduate: row 0 in s-pair strided chunks (early
    first byte), rows 1-3 as half rows split across the sync HWDGE queue
    and the gpsimd software-DGE queue so the two drains overlap.
  - All input loads ride the sync queue ordered idx0 (gates the first
    gather), ident (gates the first transpose), idx1-3, rhs.
"""
import os
import numpy as np
from contextlib import ExitStack

SU, TU = 10000.0, 86400.0
N, M, L, E = 32, 128, 128, 64
NLOC = 4096
NCORES = 8
ROWS = N // NCORES  # 4 batch rows per core
K = 36  # 32 G^T rows + [m, m, 1, 1]

_CACHE = {}


def _install_profhook():
    """Optional: shim the missing antenv.axon_hooks so trace=True works."""
    import sys
    import types
    if "antenv.axon_hooks" in sys.modules:
        return True
    try:
        from trn_agent_boot.trn_boot import _ntff_profile_via_ctypes
    except Exception:
        return False
    hook = [None]
    mod = types.ModuleType("antenv.axon_hooks")
    mod.set_axon_ntff_profile_hook = lambda h: hook.__setitem__(0, h)
    mod.get_axon_ntff_profile_hook = lambda: hook[0]
    sys.modules["antenv.axon_hooks"] = mod
    try:
        mod.set_axon_ntff_profile_hook(
            _ntff_profile_via_ctypes("/opt/axon/libaxon_pjrt.so"))
    except Exception:
        return False
    return True


def _build():
    import concourse.bass as bass
    import concourse.tile as tile
    from concourse import bacc, mybir

    F32 = mybir.dt.float32
    BF16 = mybir.dt.bfloat16
    I32 = mybir.dt.int32

    nc = bacc.Bacc("TRN2", target_bir_lowering=False, debug=False,
                   enable_asserts=True, num_devices=NCORES)
    m2e_d = nc.dram_tensor("m2e", [NLOC + 1, 4 * K], BF16,
                           kind="ExternalInput").ap()
    idx_d = nc.dram_tensor("idx", [ROWS, M], I32, kind="ExternalInput").ap()
    rhs_d = nc.dram_tensor("rhs", [2 * K, 4 * 512], BF16,
                           kind="ExternalInput").ap()
    ident_d = nc.dram_tensor("ident", [128, 128], BF16,
                             kind="ExternalInput").ap()
    # out column = 2048*lg + 1024*h + c, declared with split dims so DMA
    # APs are plain slices
    out_d = nc.dram_tensor("out", [ROWS, M, 4, 2, 1024], BF16,
                           kind="ExternalOutput").ap()

    with tile.TileContext(nc) as tc, ExitStack() as ctx:
        const = ctx.enter_context(tc.tile_pool(name="const", bufs=1))
        ipool = ctx.enter_context(tc.tile_pool(name="idxp", bufs=ROWS))
        gpool = ctx.enter_context(tc.tile_pool(name="gath", bufs=ROWS))
        lpool = ctx.enter_context(tc.tile_pool(name="lhs", bufs=3))
        opool = ctx.enter_context(tc.tile_pool(name="orow", bufs=2))
        pst = ctx.enter_context(tc.tile_pool(name="pst", bufs=2, space="PSUM"))
        pso = ctx.enter_context(tc.tile_pool(name="pso", bufs=3, space="PSUM"))

        # input loads all on the sync queue (scalar/ACT is evict-critical;
        # sync is otherwise idle): idx0 first (gates the first gather),
        # then ident (gates the first transpose), then the rest.
        its = []
        for i in range(ROWS):
            it = ipool.tile([128, 1], I32, name=f"it{i}")
            its.append(it)
        nc.sync.dma_start(its[0][:], idx_d[0, :, None])
        ident = const.tile([128, 128], BF16)
        nc.sync.dma_start(ident[:], ident_d[:])
        for i in range(1, ROWS):
            nc.sync.dma_start(its[i][:], idx_d[i, :, None])
        # rhs rides the scalar queue: ACT is idle until its first eviction
        # (~15us), and this keeps rhs off the idx0->gather critical chain
        rhs = const.tile([64 + K, 4 * 512], BF16, tag="rhs")
        nc.scalar.dma_start(rhs[0:K, :], rhs_d[0:K])
        nc.scalar.dma_start(rhs[64:64 + K, :], rhs_d[K:2 * K])

        # all four gathers upfront (gpsimd SWDGE); 2-D out AP required
        g4s = []
        for i in range(ROWS):
            g4 = gpool.tile([128, 4 * K], BF16)
            nc.gpsimd.indirect_dma_start(
                out=g4[:], out_offset=None, in_=m2e_d[:],
                in_offset=bass.IndirectOffsetOnAxis(ap=its[i][:], axis=0))
            g4s.append(g4)

        def transpose_pair(i, q):
            """Transpose l-groups (2q, 2q+1) of row i into one lhsT tile:
            even block at partitions 0:36 free 0:128, odd at 64:100
            free 128:256 (= PE row strips 0-63 / 64-127)."""
            pt = pst.tile([128, 256], BF16)
            nc.tensor.transpose(out=pt[0:K, 0:128],
                                in_=g4s[i][:, 2 * q * K:(2 * q + 1) * K],
                                identity=ident[:])
            nc.tensor.transpose(out=pt[64:64 + K, 128:256],
                                in_=g4s[i][:, (2 * q + 1) * K:(2 * q + 2) * K],
                                identity=ident[:])
            lq = lpool.tile([128, 256], BF16)
            nc.vector.tensor_copy(out=lq[:], in_=pt[:])
            return lq

        # eviction rotation: 32 slots, DVE on 15 (it also stages the lhsT
        # copies), ACT on 17
        vslots = {0, 2, 4, 6, 9, 11, 13, 15, 17, 19, 22, 24, 26, 28, 30}
        ev_i = 0

        lq_next = transpose_pair(0, 0)
        for i in range(ROWS):
            # orow[p, lg, s, c] with out column = 2048*lg + 512*s + c
            orow = opool.tile([128, 4, 4, 512], BF16)
            for q in range(2):
                lq = lq_next
                if not (i == ROWS - 1 and q == 1):
                    ni, nq = (i, 1) if q == 0 else (i + 1, 0)
                    lq_next = transpose_pair(ni, nq)
                for s in range(4):
                    po = pso.tile([128, 1024], F32, tag="po")
                    nc.tensor.matmul(po[:, 0:512], lhsT=lq[0:K, 0:128],
                                     rhs=rhs[0:K, 512 * s:512 * (s + 1)],
                                     start=True, stop=True)
                    nc.tensor.matmul(po[:, 512:1024],
                                     lhsT=lq[64:64 + K, 128:256],
                                     rhs=rhs[64:64 + K, 512 * s:512 * (s + 1)],
                                     start=True, stop=True)
                    dst = orow[:, 2 * q:2 * q + 2, s, :]
                    if ev_i in vslots:
                        nc.vector.tensor_copy(out=dst, in_=po[:])
                    else:
                        nc.scalar.copy(out=dst, in_=po[:])
                    ev_i += 1
                    # row 0: output DMA per s-pair (earliest first byte);
                    # strided two-block DRAM AP
                    if i == 0 and s % 2 == 1:
                        h = s // 2
                        nc.sync.dma_start(
                            out_d[i][:, 2 * q:2 * q + 2, h, :],
                            orow[:, 2 * q:2 * q + 2, 2 * h:2 * h + 2, :])
                # rows 1-3: half-row chunks after each pair-iter, split
                # across two queues so the drains overlap. The EARLY lg01
                # halves ride the slow gpsimd software queue; the late lg23
                # halves (which set the kernel tail) ride the sync HWDGE
                # queue. Row 3's final chunk is split so the very last DMA
                # is only 512KB.
                if i >= 1 and q == 0:
                    nc.gpsimd.dma_start(
                        out_d[i][:, 0:2, :, :], orow[:, 0:2, :, :])
                elif i >= 1 and q == 1:
                    if i < ROWS - 1:
                        nc.sync.dma_start(
                            out_d[i][:, 2:4, :, :], orow[:, 2:4, :, :])
                    else:
                        # final chunk split in two so the very last DMA is
                        # only 512KB (both on the fast sync queue; the
                        # scalar queue drains far slower)
                        for h in range(2):
                            nc.sync.dma_start(
                                out_d[i][:, 2:4, h, :],
                                orow[:, 2:4, 2 * h:2 * h + 2, :])
    nc.compile()
    return nc


def kernel(traj_loc, mat2, vec, traj_len, l_max, emb_sl_w, emb_su_w,
           emb_tl_w, emb_tu_w):
    import ml_dtypes
    from concourse import bass_utils

    BF = ml_dtypes.bfloat16
    traj_loc = np.asarray(traj_loc).astype(np.int64)
    mat2 = np.ascontiguousarray(np.asarray(mat2, dtype=np.float32))
    traj_len = np.asarray(traj_len).astype(np.int64)
    esl = np.asarray(emb_sl_w, dtype=np.float32)
    esu = np.asarray(emb_su_w, dtype=np.float32)
    etl = np.asarray(emb_tl_w, dtype=np.float32)
    etu = np.asarray(emb_tu_w, dtype=np.float32)

    # host prep: constants
    A = esl + etl                                            # [2, E]
    B = (esu - esl) / np.float32(SU) + (etu - etl) / np.float32(TU)
    mask = (np.arange(M)[None, :] < traj_len[:, None])       # [N, M]
    idx_full = np.where(mask, traj_loc - 1, NLOC).astype(np.int32)

    def split(x):
        hi = x.astype(BF)
        lo = (x - hi.astype(np.float32)).astype(BF)
        return hi, lo

    b1 = B[1].astype(BF)
    dA = A[1] - A[0]
    dAhi, dAlo = split(dA)
    a0hi, a0lo = split(A[0])

    # gather table [4097, 4, 36]: per l-group 32 mat2 columns + [m, m, 1, 1].
    # Invalid positions index the appended row 4096: zeros + [0, 0, 1, 1].
    m2e = np.zeros((NLOC + 1, 4, K), BF)
    m2bf = mat2.astype(BF)
    for lg in range(4):
        m2e[:NLOC, lg, 0:32] = m2bf[:, 32 * lg:32 * (lg + 1)]
    m2e[:NLOC, :, 32] = 1
    m2e[:NLOC, :, 33] = 1
    m2e[:, :, 34] = 1
    m2e[:, :, 35] = 1

    # rhs [72, 2048]: rows 0-35 = patterns (row 8s+lp scales e-block lp of
    # s-block by B1; rows 32-35 = dAhi/dAlo/A0hi/A0lo tiled); rows 36-71 =
    # the same block again (loads at partition 64 for the odd-tile matmuls).
    rhs = np.zeros((2 * K, 4 * 512), BF)
    for s in range(4):
        for lp in range(8):
            rhs[8 * s + lp, 512 * s + E * lp:512 * s + E * (lp + 1)] = b1
        rhs[32, 512 * s:512 * (s + 1)] = np.tile(dAhi, 8)
        rhs[33, 512 * s:512 * (s + 1)] = np.tile(dAlo, 8)
        rhs[34, 512 * s:512 * (s + 1)] = np.tile(a0hi, 8)
        rhs[35, 512 * s:512 * (s + 1)] = np.tile(a0lo, 8)
    rhs[K:2 * K] = rhs[0:K]
    ident = np.eye(128, dtype=np.float32).astype(BF)

    if "nc" not in _CACHE:
        _CACHE["nc"] = _build()
    nc = _CACHE["nc"]

    in_maps = []
    for c in range(NCORES):
        sl = slice(ROWS * c, ROWS * (c + 1))
        in_maps.append({
            "m2e": m2e.reshape(NLOC + 1, 4 * K),
            "idx": np.ascontiguousarray(idx_full[sl]),
            "rhs": rhs,
            "ident": ident,
        })

    trace = os.environ.get("KERNEL_TRACE", "0") == "1" and _install_profhook()
    res = bass_utils.run_bass_kernel_spmd(
        nc, in_maps, core_ids=list(range(NCORES)), trace=bool(trace))
    if trace:
        _CACHE["exec_time_ns"] = res.exec_time_ns
        _CACHE["trace_path"] = (res.instructions_and_trace or (None, None))[1]
        _CACHE["tmpdir"] = res.profile_json

    out = np.concatenate(
        [res.results[c]["out"].reshape(ROWS, M, L, E) for c in range(NCORES)],
        axis=0).astype(np.float32)
    return out
